# revision 33
# baseline (speedup 1.0000x reference)
"""Trainium2 Bass kernel for a GPT-2-style transformer block.

Problem: x[4,2048,768] through pre-LN attention (12 heads, causal) + pre-LN MLP
(4x hidden, tanh-approx gelu), residual connections.

Sharding: 8 cores = 4 batch elements x 2-way tensor parallel (heads 0-5 / 6-11
for attention, hidden cols 0-1535 / 1536-3071 for the MLP). Pairwise AllReduce
{0,1}{2,3}{4,5}{6,7} after c_proj and after c_fc2, chunked by token blocks.

Device layout is feature-major ([C, T]: features on partitions, tokens on the
free dim). The host pre-transposes x, pre-folds LN gains/biases into the weight
matrices, and transposes the output back. LN partition-dim sums are matmuls
against a [128,128] ones matrix, which broadcasts the sums to every partition
for free; all stat math then runs on wide [128,TN] tiles. Softmax uses exp
without max subtraction (logits are O(1) here) with denominators recovered
through an appended ones-column on V, a fast approximate reciprocal, and a
partition broadcast applied to the small attention output.

The whole kernel lives in one SBUF pool with shared tags so MLP chunks can be
emitted interleaved between attention q-chunks: every AllReduce gets tens of
microseconds of independent matmul work emitted between its issue and its
first consumer, which keeps the in-order engines from ever stalling on a
collective. Attention emits each head's S matmuls as one dense block and the
P@V accumulation as a second block so the exp/mask chain never bubbles the PE.
Matmuls run in bf16 (fp32 PSUM accumulation); the residual stream stays fp32.

Collective payloads are staged bf16 in partition-major [128, 6*W] layout: half
the wire bytes and one 128-descriptor DMA per stage/unstage. LN normalize ops
run in-place on the bf16 x-copies (DVE 2x perf mode); qkv bias-adds and v
copies ride the scalar engine's Copy activation instead of the DVE.
"""

import numpy as np
import ml_dtypes

import concourse.bacc as bacc
import concourse.bass as bass
import concourse.mybir as mybir
import concourse.tile as tile
from concourse.bass_utils import run_bass_kernel_spmd

N_CORES = 8
B, T, C = 4, 2048, 768
H = 12
HD = 64
HIDDEN = 4 * C
LN_EPS = 1e-5

NC_CHUNKS = C // 128          # 6 feature chunks
TC = 4                        # token chunks
TN = T // TC                  # 512 tokens per chunk
KT = T // 128                 # 16 k-subtiles
H_LOC = H // 2                # 6 heads per core
QKW = H_LOC * HD              # 384 per-core q/k/v width
HID_LOC = HIDDEN // 2         # 1536 per-core hidden
SCALE = 1.0 / 8.0             # 1/sqrt(64)

F32 = mybir.dt.float32
BF16 = mybir.dt.bfloat16

REPLICA_GROUPS = [[0, 1], [2, 3], [4, 5], [6, 7]]


def _build_nc():
    nc = bacc.Bacc("TRN2", target_bir_lowering=False, debug=False,
                   num_devices=N_CORES)

    x_in = nc.dram_tensor("x_fm", [C, T], F32, kind="ExternalInput")
    wqk = nc.dram_tensor("wqk", [C, 2 * QKW], BF16, kind="ExternalInput")
    wv = nc.dram_tensor("wv", [C, QKW], BF16, kind="ExternalInput")
    wproj = nc.dram_tensor("wproj", [QKW, C], BF16, kind="ExternalInput")
    wfc = nc.dram_tensor("wfc", [C, HID_LOC], BF16, kind="ExternalInput")
    wfc2 = nc.dram_tensor("wfc2", [HID_LOC, C], BF16, kind="ExternalInput")
    bqk_d = nc.dram_tensor("bqk", [128, 6], F32, kind="ExternalInput")
    pbias_d = nc.dram_tensor("pbias", [128, 6], F32, kind="ExternalInput")
    bproj_d = nc.dram_tensor("bproj", [128, 6], F32, kind="ExternalInput")
    bfc_d = nc.dram_tensor("bfc", [128, 12], F32, kind="ExternalInput")
    bfc2_d = nc.dram_tensor("bfc2", [128, 6], F32, kind="ExternalInput")
    out_d = nc.dram_tensor("out_fm", [C, T], F32, kind="ExternalOutput")

    with tile.TileContext(nc) as tc_:
        _emit(nc, tc_, x_in, wqk, wv, wproj, wfc, wfc2,
              bqk_d, pbias_d, bproj_d, bfc_d, bfc2_d, out_d)

    nc.compile()
    return nc


def _emit(nc, tc_, x_in, wqk, wv, wproj, wfc, wfc2,
          bqk_d, pbias_d, bproj_d, bfc_d, bfc2_d, out_d):
    ts = bass.ts

    pool = tc_.alloc_tile_pool(name="main", bufs=1)
    psum = tc_.alloc_tile_pool(name="psum", bufs=1, space="PSUM")
    dram = tc_.alloc_tile_pool(name="dram", bufs=1, space="DRAM")

    # ---- persistent tensors ----
    x_tiles = []
    for c in range(NC_CHUNKS):
        xt = pool.tile([128, T], F32, tag=f"x{c}", name=f"x{c}")
        x_tiles.append(xt)
    # t-major load order: chunk-0 stats can start after ~1/4 of the bytes
    for tcix in range(TC):
        for c in range(NC_CHUNKS):
            nc.sync.dma_start(out=x_tiles[c][:, ts(tcix, TN)],
                              in_=x_in.ap()[ts(c, 128), ts(tcix, TN)])

    ones_m = pool.tile([128, 128], BF16, tag="ones_m", name="ones_m")
    nc.vector.memset(ones_m[:], 1.0)
    eps_t = pool.tile([128, 1], F32, tag="eps_t", name="eps_t")
    nc.vector.memset(eps_t[:], LN_EPS)

    def load_bias(dram_t, cols, nm):
        t = pool.tile([128, cols], F32, tag=nm, name=nm)
        nc.sync.dma_start(out=t[:], in_=dram_t.ap())
        return t

    bqk_sb = load_bias(bqk_d, 6, "bqk_sb")
    pbias_sb = load_bias(pbias_d, 6, "pbias_sb")
    bproj_sb = load_bias(bproj_d, 6, "bproj_sb")
    bfc_sb = load_bias(bfc_d, 12, "bfc_sb")
    bfc2_sb = load_bias(bfc2_d, 6, "bfc2_sb")

    def load_w(dram_t, nchunks, width, nm):
        out = []
        for c in range(nchunks):
            t = pool.tile([128, width], BF16, tag=f"{nm}{c}", name=f"{nm}{c}")
            nc.sync.dma_start(out=t[:], in_=dram_t.ap()[ts(c, 128), :])
            out.append(t)
        return out

    wqk_sb = load_w(wqk, NC_CHUNKS, 2 * QKW, "wqk")
    wv_sb = load_w(wv, NC_CHUNKS, QKW, "wv")
    wproj_sb = load_w(wproj, 3, C, "wpj")
    wfc_sb = load_w(wfc, NC_CHUNKS, HID_LOC, "wfc")
    wfc2_sb = load_w(wfc2, 12, C, "wfc2")

    # AllReduce bounce buffers (per token chunk). Partition-major [128, 6*W]
    # bf16: one 128-descriptor DMA per stage/unstage and half the collective
    # wire bytes vs the old [768, W] fp32 layout.
    ar1_in = [dram.tile([128, 6 * TN], BF16, tag=f"ar1i{t}", name=f"ar1i{t}")
              for t in range(TC)]
    ar1_out = [dram.tile([128, 6 * TN], BF16, tag=f"ar1o{t}", name=f"ar1o{t}")
               for t in range(TC)]
    ar2_in = [dram.tile([128, 6 * TN], BF16, tag=f"ar2i{t}", name=f"ar2i{t}")
              for t in range(TC)]
    ar2_out = [dram.tile([128, 6 * TN], BF16, tag=f"ar2o{t}", name=f"ar2o{t}")
               for t in range(TC)]
    # asymmetric last-chunk split: the final collective carries only 128
    # tokens so the post-PE tail is as short as possible
    Q3A, Q3B = 384, 128
    ar3a = (dram.tile([128, 6 * Q3A], BF16, tag="ar3ai", name="ar3ai"),
            dram.tile([128, 6 * Q3A], BF16, tag="ar3ao", name="ar3ao"))
    ar3b = (dram.tile([128, 6 * Q3B], BF16, tag="ar3bi", name="ar3bi"),
            dram.tile([128, 6 * Q3B], BF16, tag="ar3bo", name="ar3bo"))

    # tiny warmup collective: absorbs ncfw first-collective latency during
    # the LN1/qkv phase
    warm_in = dram.tile([1, 128], F32, tag="warm_i", name="warm_i")
    warm_out = dram.tile([1, 128], F32, tag="warm_o", name="warm_o")
    wtile = pool.tile([1, 128], F32, tag="warm_t", bufs=1, name="warm_t")
    nc.vector.memset(wtile[:], 0.0)
    nc.sync.dma_start(out=warm_in[:], in_=wtile[:])
    nc.gpsimd.collective_compute(
        "AllReduce", mybir.AluOpType.add, replica_groups=REPLICA_GROUPS,
        ins=[warm_in.opt()], outs=[warm_out.opt()])

    # attention working set. q/k and v live in fp8e4m3: the matmul dtype is
    # valid, the logit/cv error it adds is ~1e-3 relative, and the freed SBUF
    # pays for a 32-deep P-tile pool (two heads in flight).
    FP8 = mybir.dt.float8e4
    qk_sb = [pool.tile([128, T], FP8, tag=f"qk{i}", name=f"qk{i}")
             for i in range(6)]
    vaug = [pool.tile([128, H_LOC * (HD + 1)], FP8, tag=f"va{i}", name=f"va{i}")
            for i in range(KT)]
    cvt_sb = [pool.tile([128, T], BF16, tag=f"cvt{i}", name=f"cvt{i}")
              for i in range(3)]

    # ---- reusable op blocks ----
    def ln_sums(tcix, tag):
        """bf16 copy of x + squares, stat matmuls. Returns psums + xr tiles;
        make_h later normalizes the xr tiles IN PLACE (the stat matmuls are
        their only raw-x readers, and mu/rstd already depend on them)."""
        tsl = ts(tcix, TN)
        sum_ps = psum.tile([128, TN], F32, tag="mmps", bufs=3,
                           name=f"{tag}sum_ps")
        ssq_ps = psum.tile([128, TN], F32, tag="mmps", bufs=3,
                           name=f"{tag}ssq_ps")
        xrs = []
        for c in range(NC_CHUNKS):
            xr = pool.tile([128, TN], BF16, tag="xr", bufs=12, name=f"{tag}xr")
            nc.vector.tensor_copy(xr[:], x_tiles[c][:, tsl])
            sq = pool.tile([128, TN], BF16, tag="sq", bufs=1, name=f"{tag}sq")
            nc.vector.tensor_mul(sq[:], xr[:], xr[:])
            nc.tensor.matmul(sum_ps[:], ones_m[:], xr[:],
                             start=(c == 0), stop=(c == NC_CHUNKS - 1))
            nc.tensor.matmul(ssq_ps[:], ones_m[:], sq[:],
                             start=(c == 0), stop=(c == NC_CHUNKS - 1))
            xrs.append(xr)
        return sum_ps, ssq_ps, xrs

    def ln_finish(sums, tag):
        """mu/rstd as broadcast [128,TN] tiles, in bf16 so the normalize
        ops run in the DVE 2x perf mode."""
        sum_ps, ssq_ps, xrs = sums
        mu16 = pool.tile([128, TN], BF16, tag="stmu", bufs=2, name=f"{tag}mu")
        nc.vector.tensor_scalar_mul(mu16[:], sum_ps[:], 1.0 / C)
        msq = pool.tile([128, TN], F32, tag="stmsq", bufs=1, name=f"{tag}msq")
        nc.vector.tensor_mul(msq[:], mu16[:], mu16[:])
        nc.vector.scalar_tensor_tensor(
            out=msq[:], in0=ssq_ps[:], scalar=1.0 / C, in1=msq[:],
            op0=mybir.AluOpType.mult, op1=mybir.AluOpType.subtract)
        nc.scalar.activation(out=msq[:], in_=msq[:],
                             func=mybir.ActivationFunctionType.Sqrt,
                             bias=eps_t[:, :])
        rstd_f = pool.tile([128, TN], F32, tag="strsf", bufs=1,
                           name=f"{tag}rstdf")
        nc.vector.reciprocal_approx_fast(out=rstd_f[:], in_=msq[:])
        rstd16 = pool.tile([128, TN], BF16, tag="strs", bufs=2,
                           name=f"{tag}rstd")
        nc.vector.tensor_copy(rstd16[:], rstd_f[:])
        return mu16, rstd16, xrs

    def make_h(mu16, rstd16, xrs):
        """In-place normalize: xr <- (xr - mu) * rstd, all-bf16 DVE ops."""
        for xr in xrs:
            nc.vector.tensor_sub(xr[:], xr[:], mu16[:])
            nc.vector.tensor_mul(xr[:], xr[:], rstd16[:])
        return xrs

    def qk_chain(tcix, hp, oc):
        tsl = ts(tcix, TN)
        ps = psum.tile([128, TN], F32, tag="mmps", bufs=3, name="qkps")
        for c in range(NC_CHUNKS):
            nc.tensor.matmul(ps[:], wqk_sb[c][:, ts(oc, 128)], hp[c][:],
                             start=(c == 0), stop=(c == NC_CHUNKS - 1))
        nc.vector.tensor_scalar_add(qk_sb[oc][:, tsl], ps[:],
                                    bqk_sb[:, oc:oc + 1])

    def v_chain(tcix, hp, s4):
        kt = tcix * 4 + s4
        vps = psum.tile([128, QKW], F32, tag="mmps", bufs=3, name="vps")
        for c in range(NC_CHUNKS):
            nc.tensor.matmul(vps[:], hp[c][:, ts(s4, 128)], wv_sb[c][:],
                             start=(c == 0), stop=(c == NC_CHUNKS - 1))
        va = vaug[kt]
        va_v = va[:].rearrange("p (h d) -> p h d", h=H_LOC)[:, :, 0:HD]
        nc.vector.tensor_copy(va_v, vps[:].rearrange("p (h d) -> p h d",
                                                     h=H_LOC))
        va_o = va[:].rearrange("p (h d) -> p h d", h=H_LOC)[:, :, HD:HD + 1]
        nc.vector.memset(va_o, 1.0)

    def qkv_pieces(tcix, hp):
        """qkv as a list of independent matmul-chain closures, usable as
        PE filler between exp-gated attention heads."""
        return ([lambda oc=oc: qk_chain(tcix, hp, oc) for oc in range(6)] +
                [lambda s4=s4: v_chain(tcix, hp, s4) for s4 in range(TC)])

    def qkv_block(tcix, hp):
        for p in qkv_pieces(tcix, hp):
            p()

    def attention_qc(qc, heads=None, fillers=None, defer_proj=False):
        """S and PV zipped 1:1 across adjacent heads: head h's exp-gated S
        matmuls interleave with head h-1's ungated PV matmuls, so the PE
        never idles a full exp latency between matmuls (which kept
        retriggering the HAM clock throttle)."""
        qsl = ts(qc, TN)
        n_kc = 4 * (qc + 1)

        def emit_S(h, kc):
            poff = (h % 2) * 64
            sps = psum.tile([128, TN], F32, tag="sps", bufs=3, name="sps")
            nc.tensor.matmul(sps[:],
                             qk_sb[3 + h // 2][poff:poff + 64, ts(kc, 128)],
                             qk_sb[h // 2][poff:poff + 64, qsl],
                             start=True, stop=True)
            pt = pool.tile([128, TN], BF16, tag="ptg", bufs=20, name="pt")
            nc.scalar.activation(out=pt[:], in_=sps[:],
                                 func=mybir.ActivationFunctionType.Exp,
                                 scale=SCALE)
            j = kc - 4 * qc
            if j >= 0:
                w = j * 128 + 128
                nc.gpsimd.affine_select(
                    out=pt[:, 0:w], in_=pt[:, 0:w],
                    pattern=[[1, w]],
                    compare_op=mybir.AluOpType.is_ge,
                    fill=0.0, base=-j * 128, channel_multiplier=-1)
            return pt

        def emit_PV(h, kc, pts, cvps):
            nc.tensor.matmul(cvps[:], vaug[kc][:, ts(h, HD + 1)],
                             pts[kc][:],
                             start=(kc == 0), stop=(kc == n_kc - 1))

        def finish_head(h, cvps):
            rd = pool.tile([1, TN], F32, tag="rd", bufs=2, name="rd")
            nc.vector.tensor_copy(rd[:], cvps[HD:HD + 1, :])
            nc.vector.reciprocal_approx_fast(out=rd[:], in_=rd[:])
            db = pool.tile([64, TN], F32, tag="db", bufs=1, name="db")
            nc.gpsimd.partition_broadcast(db[:], rd[:])
            poff = (h % 2) * 64
            nc.vector.tensor_mul(cvt_sb[h // 2][poff:poff + 64, qsl],
                                 cvps[0:HD, :], db[:])

        fill = list(fillers) if fillers else []
        hl = list(heads if heads is not None else range(H_LOC))
        per = (len(fill) + len(hl) - 1) // len(hl) if fill else 0
        prev = None  # (head, pts, cvps) pending PV
        for h in hl:
            pts = []
            cvps = psum.tile([HD + 1, TN], F32, tag="cvps", bufs=2,
                             name="cvps")
            for kc in range(n_kc):
                pts.append(emit_S(h, kc))
                if prev is not None:
                    emit_PV(prev[0], kc, prev[1], prev[2])
            if prev is not None:
                finish_head(prev[0], prev[2])
            prev = (h, pts, cvps)
            # dense filler matmul chains keep the PE busy (and the HAM
            # clock-gate warm) while this head's exp chain drains on ACT
            for _ in range(per):
                if fill:
                    fill.pop(0)()
        for kc in range(n_kc):
            emit_PV(prev[0], kc, prev[1], prev[2])
        finish_head(prev[0], prev[2])
        while fill:
            fill.pop(0)()
        if heads is not None and heads[-1] != H_LOC - 1:
            return None

        # proj + its AllReduce, as pieces so the caller can defer them into
        # the next q-chunk's head-gaps as ACT-free PE filler
        stg_box = []

        def proj_piece(oc):
            if not stg_box:
                stg_box.append(pool.tile([128, 6 * TN], BF16, tag="stg",
                                         bufs=1, name="stg"))
            pps = psum.tile([128, TN], F32, tag="mmps", bufs=3, name="pps")
            for c3 in range(3):
                nc.tensor.matmul(pps[:], wproj_sb[c3][:, ts(oc, 128)],
                                 cvt_sb[c3][:, qsl],
                                 start=(c3 == 0), stop=(c3 == 2))
            nc.vector.tensor_scalar_add(stg_box[0][:, ts(oc, TN)], pps[:],
                                        pbias_sb[:, oc:oc + 1])

        def proj_fire():
            nc.sync.dma_start(out=ar1_in[qc][:], in_=stg_box[0][:])
            nc.gpsimd.collective_compute(
                "AllReduce", mybir.AluOpType.add,
                replica_groups=REPLICA_GROUPS,
                ins=[ar1_in[qc].opt()], outs=[ar1_out[qc].opt()])

        pieces = ([lambda oc=oc: proj_piece(oc) for oc in range(NC_CHUNKS)] +
                  [proj_fire])
        if defer_proj:
            return pieces
        for p in pieces:
            p()
        return None

    def mlp_pre(tcix):
        """residual 1 + LN2 + h2 for one chunk (requires ar1_out[tcix]).

        The residual update, bf16 copy/square, and stat matmuls are emitted
        per feature chunk so the PE's first stat matmul only waits on one
        residual op, not six."""
        tsl = ts(tcix, TN)
        sum_ps = psum.tile([128, TN], F32, tag="mmps", bufs=3, name="l2sum_ps")
        ssq_ps = psum.tile([128, TN], F32, tag="mmps", bufs=3, name="l2ssq_ps")
        # unstage on the ACT hwdge queue: the sync queue head can be blocked
        # by a staging DMA whose producer hasn't finished yet
        art = pool.tile([128, 6 * TN], BF16, tag="art", bufs=1, name="art")
        nc.scalar.dma_start(out=art[:], in_=ar1_out[tcix][:])
        xrs = []
        for c in range(NC_CHUNKS):
            nc.vector.scalar_tensor_tensor(
                out=x_tiles[c][:, tsl], in0=art[:, ts(c, TN)],
                scalar=bproj_sb[:, c:c + 1], in1=x_tiles[c][:, tsl],
                op0=mybir.AluOpType.add, op1=mybir.AluOpType.add)
            xr = pool.tile([128, TN], BF16, tag="xr", bufs=12, name="l2xr")
            nc.vector.tensor_copy(xr[:], x_tiles[c][:, tsl])
            sq = pool.tile([128, TN], BF16, tag="sq", bufs=1, name="l2sq")
            nc.vector.tensor_mul(sq[:], xr[:], xr[:])
            nc.tensor.matmul(sum_ps[:], ones_m[:], xr[:],
                             start=(c == 0), stop=(c == NC_CHUNKS - 1))
            nc.tensor.matmul(ssq_ps[:], ones_m[:], sq[:],
                             start=(c == 0), stop=(c == NC_CHUNKS - 1))
            xrs.append(xr)
        mu16, rstd16, xrs = ln_finish((sum_ps, ssq_ps, xrs), "l2")
        return make_h(mu16, rstd16, xrs)

    def fc1_chain(hp, oc, g_tiles, lo=0, width=TN):
        ps = psum.tile([128, TN], F32, tag="mmps", bufs=3, name="fcps")
        for c in range(NC_CHUNKS):
            nc.tensor.matmul(ps[:, 0:width],
                             wfc_sb[c][:, ts(oc, 128)],
                             hp[c][:, lo:lo + width],
                             start=(c == 0), stop=(c == NC_CHUNKS - 1))
        g = pool.tile([128, TN], BF16, tag="gt", bufs=14, name="g")
        nc.scalar.activation(
            out=g[:, 0:width], in_=ps[:, 0:width],
            func=mybir.ActivationFunctionType.Gelu_apprx_tanh,
            bias=bfc_sb[:, oc:oc + 1])
        g_tiles.append(g)

    def mlp_fc(tcix, hp, lo=0, width=TN, ar=None, g_tiles=None):
        if g_tiles is None:
            g_tiles = []
        for oc in range(12 - len(g_tiles)):
            fc1_chain(hp, len(g_tiles), g_tiles, lo, width)
        stg = pool.tile([128, 6 * TN], BF16, tag="stg", bufs=1, name="stg2")
        for oc in range(NC_CHUNKS):
            ps = psum.tile([128, TN], F32, tag="sps", bufs=3, name="f2ps")
            for c in range(12):
                nc.tensor.matmul(ps[:, 0:width],
                                 wfc2_sb[c][:, ts(oc, 128)],
                                 g_tiles[c][:, 0:width],
                                 start=(c == 0), stop=(c == 11))
            nc.vector.tensor_copy(stg[:, oc * width:(oc + 1) * width],
                                  ps[:, 0:width])
        dst = ar[0] if ar is not None else ar2_in[tcix]
        nc.sync.dma_start(out=dst[:], in_=stg[:, 0:6 * width])
        pair = ar if ar is not None else (ar2_in[tcix], ar2_out[tcix])
        nc.gpsimd.collective_compute(
            "AllReduce", mybir.AluOpType.add,
            replica_groups=REPLICA_GROUPS,
            ins=[pair[0].opt()], outs=[pair[1].opt()])

    def res2_store(tcix, lo=0, width=TN, src=None):
        base = tcix * TN + lo
        srcbuf = src if src is not None else ar2_out[tcix]
        art = pool.tile([128, 6 * TN], BF16, tag="art", bufs=1, name="art2")
        nc.scalar.dma_start(out=art[:, 0:6 * width], in_=srcbuf[:])
        for c in range(NC_CHUNKS):
            nc.vector.scalar_tensor_tensor(
                out=x_tiles[c][:, base:base + width],
                in0=art[:, c * width:(c + 1) * width],
                scalar=bfc2_sb[:, c:c + 1],
                in1=x_tiles[c][:, base:base + width],
                op0=mybir.AluOpType.add, op1=mybir.AluOpType.add)
            nc.scalar.dma_start(out=out_d.ap()[ts(c, 128), base:base + width],
                                in_=x_tiles[c][:, base:base + width])

    # ---- emission schedule ----
    # LN1 chains run one chunk ahead of the qkv GEMMs
    hps = {}

    def ln1_chain(tcix):
        mu16, rstd16, xrs = ln_finish(ln_sums(tcix, "l1"), "l1")
        hps[tcix] = make_h(mu16, rstd16, xrs)

    ln1_chain(0)
    ln1_chain(1)
    qkv_block(0, hps.pop(0))
    ln1_chain(2)

    # small attention q-chunks first so their AllReduces fire early. The
    # next chunk's qkv chains ride inside each attention call as PE filler;
    # MLP chunks thread between later attention blocks so every collective
    # has a long runway before its first consumer. ln1(3) is emitted after
    # attn(0) so its xr buffer reuse (12-deep rotation) never puts a
    # write-after-read wait ahead of the filler chains' DVE ops.
    p0 = attention_qc(0, fillers=qkv_pieces(1, hps.pop(1)), defer_proj=True)
    ln1_chain(3)
    p1 = attention_qc(1, fillers=qkv_pieces(2, hps.pop(2)) + p0,
                      defer_proj=True)
    p2 = attention_qc(2, fillers=qkv_pieces(3, hps.pop(3)) + p1,
                      defer_proj=True)
    attention_qc(3, heads=[0, 1, 2], fillers=p2)
    h2_0 = mlp_pre(0)
    attention_qc(3, heads=[3, 4, 5])
    mlp_fc(0, h2_0)
    h2_1 = mlp_pre(1)
    h2_2 = mlp_pre(2)
    mlp_fc(1, h2_1)
    h2_3 = mlp_pre(3)
    mlp_fc(2, h2_2)
    res2_store(0)
    mlp_fc(3, h2_3, lo=0, width=Q3A, ar=ar3a)
    res2_store(1)
    mlp_fc(3, h2_3, lo=Q3A, width=Q3B, ar=ar3b)
    res2_store(2)
    res2_store(3, lo=0, width=Q3A, src=ar3a[1])
    res2_store(3, lo=Q3A, width=Q3B, src=ar3b[1])

    pool.release()
    psum.release()
    dram.release()


_NC_CACHE = None


def _get_nc():
    global _NC_CACHE
    if _NC_CACHE is None:
        _NC_CACHE = _build_nc()
    return _NC_CACHE


def _fold(v):
    return np.ascontiguousarray(v.reshape(-1, 128).T).astype(np.float32)


def _prep_core(core, x, ln1_g, ln1_b, w_attn, b_attn, w_proj, b_proj,
               ln2_g, ln2_b, w_fc, b_fc, w_fc2, b_fc2):
    b = core // 2
    tp = core % 2
    qs = slice(tp * QKW, (tp + 1) * QKW)
    ks = slice(C + tp * QKW, C + (tp + 1) * QKW)
    vs = slice(2 * C + tp * QKW, 2 * C + (tp + 1) * QKW)
    hs = slice(tp * HID_LOC, (tp + 1) * HID_LOC)

    x_fm = np.ascontiguousarray(x[b].T).astype(np.float32)

    wqk_h = np.concatenate([w_attn[:, qs], w_attn[:, ks]], axis=1)
    wqk_h = (wqk_h * ln1_g[:, None]).astype(np.float32)
    wv_h = (w_attn[:, vs] * ln1_g[:, None]).astype(np.float32)

    bqk = np.concatenate([b_attn[qs], b_attn[ks]]) + ln1_b @ np.concatenate(
        [w_attn[:, qs], w_attn[:, ks]], axis=1)
    bv = b_attn[vs] + ln1_b @ w_attn[:, vs]

    wproj_h = w_proj[tp * QKW:(tp + 1) * QKW, :]
    pbias = bv @ wproj_h                       # folded v-bias contribution
    wfc_h = (w_fc[:, hs] * ln2_g[:, None]).astype(np.float32)
    bfc = b_fc[hs] + ln2_b @ w_fc[:, hs]
    wfc2_h = w_fc2[hs, :]

    # b_proj / b_fc2 are added once per core after the AllReduce
    return {
        "x_fm": x_fm,
        "wqk": wqk_h.astype(ml_dtypes.bfloat16),
        "wv": wv_h.astype(ml_dtypes.bfloat16),
        "wproj": wproj_h.astype(ml_dtypes.bfloat16),
        "wfc": wfc_h.astype(ml_dtypes.bfloat16),
        "wfc2": wfc2_h.astype(ml_dtypes.bfloat16),
        "bqk": _fold(bqk),
        "pbias": _fold(pbias),
        "bproj": _fold(np.asarray(b_proj)),
        "bfc": _fold(np.asarray(b_fc)),
        "bfc2": _fold(np.asarray(b_fc2)),
    }


def kernel(x, ln1_g, ln1_b, w_attn, b_attn, w_proj, b_proj,
           ln2_g, ln2_b, w_fc, b_fc, w_fc2, b_fc2, _trace=False):
    args = [np.asarray(a, np.float32) for a in
            (x, ln1_g, ln1_b, w_attn, b_attn, w_proj, b_proj,
             ln2_g, ln2_b, w_fc, b_fc, w_fc2, b_fc2)]
    nc = _get_nc()
    in_maps = [_prep_core(core, *args) for core in range(N_CORES)]
    res = run_bass_kernel_spmd(nc, in_maps, list(range(N_CORES)),
                               trace=_trace)
    out = np.empty((B, T, C), np.float32)
    for b in range(B):
        out[b] = res.results[2 * b]["out_fm"].T
    kernel._last_result = res
    return out

